# revision 14
# baseline (speedup 1.0000x reference)
"""Trainium2 Bass kernel for CrossModalMultiHeadAttentionK (v3).

Per-channel 7x7 local attention on a 40x40 grid, B=2, C=256, with 1x1 convs
(q/k/v/out/fuse) and sinusoidal positional encodings. Sharding: 8 cores =
(batch b in {0,1}) x (row-quarter q in {0..3}, 10 output rows each). Each core
holds all 256 channels in SBUF layout [128 partitions, 2 channel-slots,
spatial]; no cross-core collectives.

Key structure:
 - pe folded into query/key on HOST (no pe matmuls / extra DMAs); all device
   inputs fp16 (halves DMA); fp16 outputs (host casts back to fp32).
 - j-loop grouped by window row di (7 blocks of 7 offsets):
     * fused DVE muls: q broadcast over j (stride-0 AP), k/v read through
       overlapping strided views; +1-shifted k/v copies keep odd offsets
       4B-aligned for DVE 2x fp16 mode.
     * one 5600-wide exp per row-block (amortizes ACT fixed cost).
     * ONE accumulation matmul per (row, slot, num/den): moving operand
       streams all 7 offsets (FD=2800) while the PSUM output AP has stride 0
       over j, so the bank accumulates in place. 28 matmuls total instead of
       196 (kills the per-matmul identity LDWEIGHTS serialization).
 - reciprocal_approx_fast for 1/den; chunked tail pipelined across engines.
 - no gpsimd elementwise (it shares the DVE SBUF port; measured 3.7x DVE
   slowdown under contention).
"""

import math
import numpy as np

# ---- problem constants (hardcoded per harness contract) ----
B, C, H, W = 2, 256, 40, 40
KS, PAD = 7, 3
HEAD_DIM = 32
SCALING = HEAD_DIM ** -0.5
TEMPERATURE, PESCALE, EPS = 10000.0, 2.0 * math.pi, 1e-6
NQ = 4                 # row-quarters
RQ = H // NQ           # 10 output rows per core
NPOS = RQ * W          # 400 output positions per slot
KROWS = RQ + KS - 1    # 16 padded rows needed
KW = W + 2 * PAD       # 46 padded cols
KFREE = KROWS * KW     # 736
NJ = KS * KS           # 49 window offsets
JEVEN = [0, 2, 4, 6]   # dj from unshifted buffers
JODD = [1, 3, 5]       # dj from +1-shifted buffers
NJE, NJO = len(JEVEN), len(JODD)
NF = 2 * NPOS          # 800 elems per (row, slot) metarow plane
ROWBLK = KS * NPOS     # 2800 elems per (slot, row-block)
SBLK = 2 * ROWBLK      # 5600 elems per row-block tile

_CACHE = {}


def _sine_pe(mask):
    """numpy port of reference.sine_pe; mask (b,h,w) bool."""
    nm = (~mask).astype(np.float32)
    y = np.cumsum(nm, axis=1, dtype=np.float32)
    x = np.cumsum(nm, axis=2, dtype=np.float32)
    y = y / (y[:, -1:, :] + EPS) * PESCALE
    x = x / (x[:, :, -1:] + EPS) * PESCALE
    nf = C // 2
    i = np.arange(nf, dtype=np.float32)
    dim_t = (TEMPERATURE ** (2.0 * np.floor(i / 2.0) / nf)).astype(np.float32)
    px = (x[..., None] / dim_t).astype(np.float32)
    py = (y[..., None] / dim_t).astype(np.float32)

    def interleave(p):
        s = np.stack([np.sin(p[..., 0::2]), np.cos(p[..., 1::2])], axis=4)
        return s.reshape(p.shape[0], p.shape[1], p.shape[2], -1)

    pos = np.concatenate([interleave(py), interleave(px)], axis=3)
    return pos.transpose(0, 3, 1, 2).astype(np.float32)  # (b, C, h, w)


def _pe_constants():
    if "pe" in _CACHE:
        return _CACHE["pe"]
    mask_q = np.zeros((1, H, W), dtype=bool)
    pe_q = _sine_pe(mask_q)[0]  # (C, H, W)
    Hp, Wp = H + 2 * PAD, W + 2 * PAD
    mask_k = np.zeros((1, Hp, Wp), dtype=bool)
    mask_k[:, :PAD, :] = True
    mask_k[:, :, :PAD] = True
    mask_k[:, Hp - PAD:, :] = True
    mask_k[:, :, Wp - PAD:] = True
    pe_k = _sine_pe(mask_k)[0]  # (C, Hp, Wp)
    _CACHE["pe"] = (pe_q, pe_k)
    return pe_q, pe_k


def _build_module():
    """Build (once) the per-core Bacc module. Same NEFF on all 8 cores."""
    if "nc" in _CACHE:
        return _CACHE["nc"]
    import concourse.bacc as bacc
    import concourse.tile as tile
    import concourse.mybir as mybir
    from concourse.ap import AP

    f32 = mybir.dt.float32
    f16 = mybir.dt.float16
    AF = mybir.ActivationFunctionType

    nc = bacc.Bacc("TRN2", target_bir_lowering=False, debug=False,
                   enable_asserts=True, num_devices=8)

    din = {}
    for name, shape, dt in [
        ("querype", [128, 2, NPOS], f16),
        ("keypad", [128, 2, KFREE], f16),
        ("keypadpe", [128, 2, KFREE], f16),
        ("wq", [128, 2, 256], f16),
        ("wk", [128, 2, 256], f16),
        ("wv", [128, 2, 256], f16),
        ("wo", [128, 2, 256], f16),
        ("wf", [128, 4, 256], f16),
        ("biases", [128, 8], f32),  # bq, bk, bv, bo columns (x2 each)
        ("ident", [128, 128], f16),
    ]:
        din[name] = nc.dram_tensor(name, shape, dt, kind="ExternalInput").ap()
    d_out = nc.dram_tensor("out_part", [128, 2, NPOS], f16, kind="ExternalOutput").ap()
    d_vo = nc.dram_tensor("vo_part", [128, 2, NPOS], f16, kind="ExternalOutput").ap()

    with tile.TileContext(nc) as tc:
        with tc.tile_pool(name="consts", bufs=1) as cp, \
             tc.tile_pool(name="work", bufs=1) as wp, \
             tc.tile_pool(name="sje", bufs=3) as sp, \
             tc.tile_pool(name="psacc", bufs=1, space="PSUM") as pa, \
             tc.tile_pool(name="psbig", bufs=1, space="PSUM") as pk, \
             tc.tile_pool(name="psq", bufs=2, space="PSUM") as pq:

            # ---- consolidated input DMAs: one per tensor, priority queues ----
            sb = {}

            def wtile(nm, nk):
                t = cp.tile([128, nk, 256], f16, tag=nm, name=nm)
                sb[nm] = t
                return t[:], din[nm][:]

            # critical-path first; keypadpe split across two queues
            sb["querype"] = cp.tile([128, 2, NPOS], f16, tag="querype", name="querype")
            sb["keypadpe"] = cp.tile([128, 2, KFREE], f16, tag="keypadpe", name="keypadpe")
            sb["keypad"] = cp.tile([128, 2, KFREE], f16, tag="keypad", name="keypad")
            sb["ident"] = cp.tile([128, 128], f16, tag="ident", name="ident")
            sb["biases"] = cp.tile([128, 8], f32, tag="biases", name="biases")
            # queue sync: wq, keypadpe[a0], ident, biases
            t, src = wtile("wq", 2)
            nc.sync.dma_start(out=t, in_=src)
            nc.sync.dma_start(out=sb["keypadpe"][:, 0, :], in_=din["keypadpe"][:, 0, :])
            nc.sync.dma_start(out=sb["ident"][:], in_=din["ident"][:])
            nc.sync.dma_start(out=sb["biases"][:], in_=din["biases"][:])
            # queue scalar: wk, keypadpe[a1], wv
            t, src = wtile("wk", 2)
            nc.scalar.dma_start(out=t, in_=src)
            nc.scalar.dma_start(out=sb["keypadpe"][:, 1, :], in_=din["keypadpe"][:, 1, :])
            t, src = wtile("wv", 2)
            nc.scalar.dma_start(out=t, in_=src)
            # queue gpsimd: querype, keypad, wo, wf
            nc.gpsimd.dma_start(out=sb["querype"][:], in_=din["querype"][:])
            nc.gpsimd.dma_start(out=sb["keypad"][:], in_=din["keypad"][:])
            t, src = wtile("wo", 2)
            nc.gpsimd.dma_start(out=t, in_=src)
            t, src = wtile("wf", 4)
            nc.gpsimd.dma_start(out=t, in_=src)

            bias = {"bq": sb["biases"][:, 0:2], "bk": sb["biases"][:, 2:4],
                    "bv": sb["biases"][:, 4:6], "bo": sb["biases"][:, 6:8]}

            # ---- q conv: q_b = Wq . querype + bq (prescaled) ----
            # q_b layout [r(10), a(2), c(40)]: slots interleaved row-wise so
            # one DVE mul covers both channel-slots (3 free dims).
            q_b = wp.tile([128, NF], f16, tag="q_b")
            for o in range(2):
                ps = pq.tile([128, NPOS], f32, tag="psq")
                for k in range(2):
                    nc.tensor.matmul(ps[:], sb["wq"][:, k, o * 128:(o + 1) * 128],
                                     sb["querype"][:, k, :], start=(k == 0), stop=(k == 1))
                qdst = AP(q_b[:].tensor, q_b[:].offset + o * W,
                          [[NF, 128], [2 * W, RQ], [1, W]])
                nc.scalar.activation(out=qdst, in_=ps[:],
                                     func=AF.Identity, bias=bias["bq"][:, o:o + 1])

            # ---- k conv (from keypadpe) and v conv (from keypad) ----
            k_b = wp.tile([128, 2 * KFREE], f16, tag="k_b")
            k_b1 = wp.tile([128, 2 * KFREE], f16, tag="k_b1")
            v_b = wp.tile([128, 2 * KFREE], f16, tag="v_b")
            v_b1 = wp.tile([128, 2 * KFREE], f16, tag="v_b1")
            for wname, src, bn, dest, dest1 in [
                ("wk", "keypadpe", "bk", k_b, k_b1),
                ("wv", "keypad", "bv", v_b, v_b1),
            ]:
                for o in range(2):
                    ps = pk.tile([128, 1024], f32, tag="psbig")
                    for sl in (slice(0, 512), slice(512, KFREE)):
                        for k in range(2):
                            nc.tensor.matmul(ps[:, sl], sb[wname][:, k, o * 128:(o + 1) * 128],
                                             sb[src][:, k, sl],
                                             start=(k == 0), stop=(k == 1))
                    kvdst = AP(dest[:].tensor, dest[:].offset + o * KW,
                               [[2 * KFREE, 128], [2 * KW, KROWS], [1, KW]])
                    nc.scalar.activation(out=kvdst,
                                         in_=ps[:, 0:KFREE], func=AF.Identity,
                                         bias=bias[bn][:, o:o + 1])
                # +1-shifted fp16 copy so odd window offsets stay 4B-aligned
                nc.vector.tensor_copy(dest1[:, 0:2 * KFREE - 1], dest[:, 1:2 * KFREE])

            # ---- fuse conv, querype half: accumulate early (keeps PE warm) ----
            # layout [128, 1024]: o=0 at cols 0:400 (bank0), o=1 at 512:912 (bank1)
            fuse_ps = pk.tile([128, 1024], f32, tag="psbig", name="fuse_ps")
            for o in range(2):
                for k in range(2):
                    nc.tensor.matmul(fuse_ps[:, o * 512:o * 512 + NPOS],
                                     sb["wf"][:, k, o * 128:(o + 1) * 128],
                                     sb["querype"][:, k, :],
                                     start=(k == 0), stop=False)

            # ---- attention j-loop, grouped by window row di ----
            num_ps = [pa.tile([128, NPOS], f32, tag=f"num{a}", name=f"num{a}")
                      for a in range(2)]
            den_ps = [pa.tile([128, NPOS], f32, tag=f"den{a}", name=f"den{a}")
                      for a in range(2)]

            RA = 2 * RQ   # merged (row, slot) dim: 20 metarows of 40

            def jmul(out_t, out_off, njs, kv_t, kv_off, in0=None):
                """out[j, ra, c] = in0 * kv[j-strided overlapping view];
                in0 defaults to q_b broadcast over j (stride 0)."""
                ov = AP(out_t[:].tensor, out_t[:].offset + out_off,
                        [[SBLK, 128], [NF, njs], [W, RA], [1, W]])
                kv = AP(kv_t[:].tensor, kv_t[:].offset + kv_off,
                        [[2 * KFREE, 128], [2, njs], [KW, RA], [1, W]])
                if in0 is None:
                    in0 = AP(q_b[:].tensor, q_b[:].offset,
                             [[NF, 128], [0, njs], [W, RA], [1, W]])
                nc.vector.tensor_mul(ov, in0, kv)

            def mm_rhs(src_t, a, jj):
                # e/p layout [j(7), ra(20), c(40)]: slice slot a of offset jj
                return AP(src_t[:].tensor, src_t[:].offset + jj * NF + a * W,
                          [[SBLK, 128], [2 * W, RQ], [1, W]])

            def acc_mm(ps_tile, src_t, a, di, last):
                """accumulate the 7 offsets of a row-block into one PSUM tile
                (per-j 128x400 fp16 matmuls; dense queue pipelines LDW+MM at
                ~169ns each when HAM-warm)."""
                for jj in range(KS):
                    nc.tensor.matmul(ps_tile[:], sb["ident"][:],
                                     mm_rhs(src_t, a, jj),
                                     start=(di == 0 and jj == 0),
                                     stop=(last and jj == KS - 1))

            # software-pipelined emission: the DVE/PE queues execute in order,
            # so s(di+1) is emitted BEFORE p(di) (which waits on exp(di)) to
            # keep DVE busy, and num(di-1) lands before den(di) on PE.
            s_ts, e_ts, p_ts = [], [], []

            def emit_s(di):
                s_t = sp.tile([128, SBLK], f16, tag="s", name=f"s{di}")
                e_t = sp.tile([128, SBLK], f16, tag="e", name=f"e{di}")
                s_ts.append(s_t)
                e_ts.append(e_t)
                jmul(s_t, 0, NJE, k_b, di * 2 * KW)
                jmul(s_t, NJE * NF, NJO, k_b1, di * 2 * KW)
                nc.scalar.activation(out=e_t[:], in_=s_t[:], func=AF.Exp)

            def emit_p(di):
                e_t = e_ts[di]
                p_t = sp.tile([128, SBLK], f16, tag="p", name=f"p{di}")
                p_ts.append(p_t)
                ev = AP(e_t[:].tensor, e_t[:].offset,
                        [[SBLK, 128], [NF, NJE], [W, RA], [1, W]])
                jmul(p_t, 0, NJE, v_b, di * 2 * KW, in0=ev)
                evo = AP(e_t[:].tensor, e_t[:].offset + NJE * NF,
                         [[SBLK, 128], [NF, NJO], [W, RA], [1, W]])
                jmul(p_t, NJE * NF, NJO, v_b1, di * 2 * KW, in0=evo)

            emit_s(0)
            # PE warm-keeper: dummy matmuls on s(0) bridge the exp(0) gap so
            # HAM doesn't re-throttle before the first den matmuls.
            warm_ps = pq.tile([128, NPOS], f32, tag="psq", name="warmps")
            for jj in range(KS):
                nc.tensor.matmul(warm_ps[:], sb["ident"][:],
                                 mm_rhs(s_ts[0], 0, jj),
                                 start=True, stop=True)
            for di in range(1, KS):
                emit_s(di)
                for a in range(2):
                    acc_mm(den_ps[a], e_ts[di - 1], a, di - 1, False)
                emit_p(di - 1)
                for a in range(2):
                    acc_mm(num_ps[a], p_ts[di - 1], a, di - 1, False)
            for a in range(2):
                acc_mm(den_ps[a], e_ts[KS - 1], a, KS - 1, True)
            emit_p(KS - 1)
            for a in range(2):
                acc_mm(num_ps[a], p_ts[KS - 1], a, KS - 1, True)

            # ---- tail: normalize + vo conv + fuse conv, chunked for overlap ----
            NCH = 2
            NCHF = 4
            HCF = NPOS // NCHF
            HC = NPOS // NCH
            r_t = wp.tile([128, 2, NPOS], f32, tag="r_t")
            att = wp.tile([128, 2, NPOS], f16, tag="att")
            vo16 = wp.tile([128, 2, NPOS], f16, tag="vo16")
            out16 = wp.tile([128, 2, NPOS], f16, tag="out16")
            for ch in range(NCH):
                cs = slice(ch * HC, (ch + 1) * HC)
                for a in range(2):
                    nc.vector.reciprocal_approx_fast(r_t[:, a, cs], den_ps[a][:, cs])
            for ch in range(NCH):
                cs = slice(ch * HC, (ch + 1) * HC)
                for a in range(2):
                    nc.vector.tensor_mul(att[:, a, cs], num_ps[a][:, cs], r_t[:, a, cs])
            vops = [pa.tile([128, NPOS], f32, tag=f"num{o}", name="vops")
                    for o in range(2)]
            for ch in range(NCHF):
                cs = slice(ch * HCF, (ch + 1) * HCF)
                for o in range(2):
                    for a in range(2):
                        nc.tensor.matmul(vops[o][:, cs], sb["wo"][:, a, o * 128:(o + 1) * 128],
                                         att[:, a, cs], start=(a == 0), stop=(a == 1))
                    nc.scalar.activation(out=vo16[:, o, cs], in_=vops[o][:, cs],
                                         func=AF.Identity, bias=bias["bo"][:, o:o + 1])
                for o in range(2):
                    for k in range(2):
                        nc.tensor.matmul(
                            fuse_ps[:, o * 512 + ch * HCF:o * 512 + (ch + 1) * HCF],
                            sb["wf"][:, 2 + k, o * 128:(o + 1) * 128],
                            vo16[:, k, cs], start=False, stop=(k == 1))
                    fslice = fuse_ps[:, o * 512 + ch * HCF:o * 512 + (ch + 1) * HCF]
                    if o == 0:
                        nc.scalar.copy(out16[:, o, cs], fslice)
                    else:
                        nc.vector.tensor_copy(out16[:, o, cs], fslice)
                nc.sync.dma_start(out=d_vo[:, :, cs], in_=vo16[:, :, cs])
                nc.gpsimd.dma_start(out=d_out[:, :, cs], in_=out16[:, :, cs])

    nc.compile()
    _CACHE["nc"] = nc
    return nc


def _in_maps(key, query, Wq, bq, Wk, bk, Wv, bv, Wo, bo, Wf):
    pe_q, pe_k = _pe_constants()
    keypad_full = np.pad(key, ((0, 0), (0, 0), (PAD, PAD), (PAD, PAD)))
    querype_full = (query + pe_q[None]).astype(np.float16)
    keypadpe_full = (keypad_full + pe_k[None]).astype(np.float16)
    keypad_full = keypad_full.astype(np.float16)
    def wdev(w, nk, scale=1.0):  # (out, in) -> [128, nk, 256] fp16
        return np.ascontiguousarray(
            (w.T * scale).reshape(nk, 128, 256).transpose(1, 0, 2)).astype(np.float16)

    wqT = wdev(Wq, 2, SCALING)
    wkT = wdev(Wk, 2)
    wvT = wdev(Wv, 2)
    woT = wdev(Wo, 2)
    wfT = wdev(Wf, 4)
    biases = np.stack([(bq * SCALING), bk, bv, bo], 0).reshape(4, 2, 128)
    biases = np.ascontiguousarray(biases.reshape(8, 128).T).astype(np.float32)
    ident = np.eye(128, dtype=np.float16)

    def part16(arr, npos):  # (C, rows*cols) -> (128, 2, npos) fp16
        return np.ascontiguousarray(
            arr.reshape(2, 128, npos).transpose(1, 0, 2)).astype(np.float16)

    maps = []
    for b in range(B):
        for q in range(NQ):
            r0 = RQ * q
            m = {
                "querype": part16(querype_full[b, :, r0:r0 + RQ, :].reshape(C, NPOS), NPOS),
                "keypad": part16(keypad_full[b, :, r0:r0 + KROWS, :].reshape(C, KFREE), KFREE),
                "keypadpe": part16(keypadpe_full[b, :, r0:r0 + KROWS, :].reshape(C, KFREE), KFREE),
                "wq": wqT, "wk": wkT, "wv": wvT, "wo": woT, "wf": wfT,
                "biases": biases, "ident": ident,
            }
            maps.append(m)
    return maps


def kernel(key, query, Wq, bq, Wk, bk, Wv, bv, Wo, bo, Wf, _trace=False):
    from concourse.bass_utils import run_bass_kernel_spmd

    args = [np.asarray(a, dtype=np.float32) for a in
            (key, query, Wq, bq, Wk, bk, Wv, bv, Wo, bo, Wf)]
    nc = _build_module()
    maps = _in_maps(*args)
    res = run_bass_kernel_spmd(nc, maps, list(range(8)), trace=_trace)
    _CACHE["last_res"] = res

    out = np.zeros((B, C, H, W), dtype=np.float32)
    vo = np.zeros((B, C, H, W), dtype=np.float32)
    for b in range(B):
        for q in range(NQ):
            r = res.results[b * NQ + q]
            r0 = RQ * q
            out[b, :, r0:r0 + RQ, :] = (
                r["out_part"].transpose(1, 0, 2).reshape(C, RQ, W).astype(np.float32))
            vo[b, :, r0:r0 + RQ, :] = (
                r["vo_part"].transpose(1, 0, 2).reshape(C, RQ, W).astype(np.float32))
    return out, vo


# revision 15
# speedup vs baseline: 1.0108x; 1.0108x over previous
"""Trainium2 Bass kernel for CrossModalMultiHeadAttentionK (v3).

Per-channel 7x7 local attention on a 40x40 grid, B=2, C=256, with 1x1 convs
(q/k/v/out/fuse) and sinusoidal positional encodings. Sharding: 8 cores =
(batch b in {0,1}) x (row-quarter q in {0..3}, 10 output rows each). Each core
holds all 256 channels in SBUF layout [128 partitions, 2 channel-slots,
spatial]; no cross-core collectives.

Key structure:
 - pe folded into query/key on HOST (no pe matmuls / extra DMAs); all device
   inputs fp16 (halves DMA); fp16 outputs (host casts back to fp32).
 - j-loop grouped by window row di (7 blocks of 7 offsets):
     * fused DVE muls: q broadcast over j (stride-0 AP), k/v read through
       overlapping strided views; +1-shifted k/v copies keep odd offsets
       4B-aligned for DVE 2x fp16 mode.
     * one 5600-wide exp per row-block (amortizes ACT fixed cost).
     * ONE accumulation matmul per (row, slot, num/den): moving operand
       streams all 7 offsets (FD=2800) while the PSUM output AP has stride 0
       over j, so the bank accumulates in place. 28 matmuls total instead of
       196 (kills the per-matmul identity LDWEIGHTS serialization).
 - reciprocal_approx_fast for 1/den; chunked tail pipelined across engines.
 - no gpsimd elementwise (it shares the DVE SBUF port; measured 3.7x DVE
   slowdown under contention).
"""

import math
import numpy as np

# ---- problem constants (hardcoded per harness contract) ----
B, C, H, W = 2, 256, 40, 40
KS, PAD = 7, 3
HEAD_DIM = 32
SCALING = HEAD_DIM ** -0.5
TEMPERATURE, PESCALE, EPS = 10000.0, 2.0 * math.pi, 1e-6
NQ = 4                 # row-quarters
RQ = H // NQ           # 10 output rows per core
NPOS = RQ * W          # 400 output positions per slot
KROWS = RQ + KS - 1    # 16 padded rows needed
KW = W + 2 * PAD       # 46 padded cols
KFREE = KROWS * KW     # 736
NJ = KS * KS           # 49 window offsets
JEVEN = [0, 2, 4, 6]   # dj from unshifted buffers
JODD = [1, 3, 5]       # dj from +1-shifted buffers
NJE, NJO = len(JEVEN), len(JODD)
NF = 2 * NPOS          # 800 elems per (row, slot) metarow plane
ROWBLK = KS * NPOS     # 2800 elems per (slot, row-block)
SBLK = 2 * ROWBLK      # 5600 elems per row-block tile

_CACHE = {}


def _sine_pe(mask):
    """numpy port of reference.sine_pe; mask (b,h,w) bool."""
    nm = (~mask).astype(np.float32)
    y = np.cumsum(nm, axis=1, dtype=np.float32)
    x = np.cumsum(nm, axis=2, dtype=np.float32)
    y = y / (y[:, -1:, :] + EPS) * PESCALE
    x = x / (x[:, :, -1:] + EPS) * PESCALE
    nf = C // 2
    i = np.arange(nf, dtype=np.float32)
    dim_t = (TEMPERATURE ** (2.0 * np.floor(i / 2.0) / nf)).astype(np.float32)
    px = (x[..., None] / dim_t).astype(np.float32)
    py = (y[..., None] / dim_t).astype(np.float32)

    def interleave(p):
        s = np.stack([np.sin(p[..., 0::2]), np.cos(p[..., 1::2])], axis=4)
        return s.reshape(p.shape[0], p.shape[1], p.shape[2], -1)

    pos = np.concatenate([interleave(py), interleave(px)], axis=3)
    return pos.transpose(0, 3, 1, 2).astype(np.float32)  # (b, C, h, w)


def _pe_constants():
    if "pe" in _CACHE:
        return _CACHE["pe"]
    mask_q = np.zeros((1, H, W), dtype=bool)
    pe_q = _sine_pe(mask_q)[0]  # (C, H, W)
    Hp, Wp = H + 2 * PAD, W + 2 * PAD
    mask_k = np.zeros((1, Hp, Wp), dtype=bool)
    mask_k[:, :PAD, :] = True
    mask_k[:, :, :PAD] = True
    mask_k[:, Hp - PAD:, :] = True
    mask_k[:, :, Wp - PAD:] = True
    pe_k = _sine_pe(mask_k)[0]  # (C, Hp, Wp)
    _CACHE["pe"] = (pe_q, pe_k)
    return pe_q, pe_k


def _build_module():
    """Build (once) the per-core Bacc module. Same NEFF on all 8 cores."""
    if "nc" in _CACHE:
        return _CACHE["nc"]
    import concourse.bacc as bacc
    import concourse.tile as tile
    import concourse.mybir as mybir
    from concourse.ap import AP

    f32 = mybir.dt.float32
    f16 = mybir.dt.float16
    AF = mybir.ActivationFunctionType

    nc = bacc.Bacc("TRN2", target_bir_lowering=False, debug=False,
                   enable_asserts=True, num_devices=8)

    din = {}
    for name, shape, dt in [
        ("querype", [128, 2, NPOS], f16),
        ("keypad", [128, 2, KFREE], f16),
        ("keypadpe", [128, 2, KFREE], f16),
        ("wq", [128, 2, 256], f16),
        ("wk", [128, 2, 256], f16),
        ("wv", [128, 2, 256], f16),
        ("wo", [128, 2, 256], f16),
        ("wf", [128, 4, 256], f16),
        ("biases", [128, 8], f32),  # bq, bk, bv, bo columns (x2 each)
        ("ident", [128, 128], f16),
    ]:
        din[name] = nc.dram_tensor(name, shape, dt, kind="ExternalInput").ap()
    d_out = nc.dram_tensor("out_part", [128, 2, NPOS], f16, kind="ExternalOutput").ap()
    d_vo = nc.dram_tensor("vo_part", [128, 2, NPOS], f16, kind="ExternalOutput").ap()

    with tile.TileContext(nc) as tc:
        with tc.tile_pool(name="consts", bufs=1) as cp, \
             tc.tile_pool(name="work", bufs=1) as wp, \
             tc.tile_pool(name="sje", bufs=3) as sp, \
             tc.tile_pool(name="psacc", bufs=1, space="PSUM") as pa, \
             tc.tile_pool(name="psbig", bufs=1, space="PSUM") as pk, \
             tc.tile_pool(name="psq", bufs=2, space="PSUM") as pq:

            # ---- consolidated input DMAs: one per tensor, priority queues ----
            sb = {}

            def wtile(nm, nk):
                t = cp.tile([128, nk, 256], f16, tag=nm, name=nm)
                sb[nm] = t
                return t[:], din[nm][:]

            # critical-path first; keypadpe split across two queues
            sb["querype"] = cp.tile([128, 2, NPOS], f16, tag="querype", name="querype")
            sb["keypadpe"] = cp.tile([128, 2, KFREE], f16, tag="keypadpe", name="keypadpe")
            sb["keypad"] = cp.tile([128, 2, KFREE], f16, tag="keypad", name="keypad")
            sb["ident"] = cp.tile([128, 128], f16, tag="ident", name="ident")
            sb["biases"] = cp.tile([128, 8], f32, tag="biases", name="biases")
            # queue sync: wq, keypadpe[a0], ident, biases
            t, src = wtile("wq", 2)
            nc.sync.dma_start(out=t, in_=src)
            nc.sync.dma_start(out=sb["keypadpe"][:, 0, :], in_=din["keypadpe"][:, 0, :])
            nc.sync.dma_start(out=sb["ident"][:], in_=din["ident"][:])
            nc.sync.dma_start(out=sb["biases"][:], in_=din["biases"][:])
            # queue scalar: wk, keypadpe[a1], wv
            t, src = wtile("wk", 2)
            nc.scalar.dma_start(out=t, in_=src)
            nc.scalar.dma_start(out=sb["keypadpe"][:, 1, :], in_=din["keypadpe"][:, 1, :])
            t, src = wtile("wv", 2)
            nc.scalar.dma_start(out=t, in_=src)
            # queue gpsimd: querype, keypad, wo, wf
            nc.gpsimd.dma_start(out=sb["querype"][:], in_=din["querype"][:])
            nc.gpsimd.dma_start(out=sb["keypad"][:], in_=din["keypad"][:])
            t, src = wtile("wo", 2)
            nc.gpsimd.dma_start(out=t, in_=src)
            t, src = wtile("wf", 4)
            nc.gpsimd.dma_start(out=t, in_=src)

            bias = {"bq": sb["biases"][:, 0:2], "bk": sb["biases"][:, 2:4],
                    "bv": sb["biases"][:, 4:6], "bo": sb["biases"][:, 6:8]}

            # ---- q conv: q_b = Wq . querype + bq (prescaled) ----
            # q_b layout [r(10), a(2), c(40)]: slots interleaved row-wise so
            # one DVE mul covers both channel-slots (3 free dims).
            q_b = wp.tile([128, NF], f16, tag="q_b")
            for o in range(2):
                ps = pq.tile([128, NPOS], f32, tag="psq")
                for k in range(2):
                    nc.tensor.matmul(ps[:], sb["wq"][:, k, o * 128:(o + 1) * 128],
                                     sb["querype"][:, k, :], start=(k == 0), stop=(k == 1))
                qdst = AP(q_b[:].tensor, q_b[:].offset + o * W,
                          [[NF, 128], [2 * W, RQ], [1, W]])
                nc.scalar.activation(out=qdst, in_=ps[:],
                                     func=AF.Identity, bias=bias["bq"][:, o:o + 1])

            # ---- k/v convs, interleaved [r(16), a(2), c(46)] output layout ----
            k_b = wp.tile([128, 2 * KFREE], f16, tag="k_b")
            k_b1 = wp.tile([128, 2 * KFREE], f16, tag="k_b1")
            v_b = wp.tile([128, 2 * KFREE], f16, tag="v_b")
            v_b1 = wp.tile([128, 2 * KFREE], f16, tag="v_b1")

            def conv_kv(wname, srcname, bn, dest, dest1):
                for o in range(2):
                    ps = pk.tile([128, 1024], f32, tag="psbig", name="kvps")
                    for sl in (slice(0, 512), slice(512, KFREE)):
                        for k in range(2):
                            nc.tensor.matmul(ps[:, sl], sb[wname][:, k, o * 128:(o + 1) * 128],
                                             sb[srcname][:, k, sl],
                                             start=(k == 0), stop=(k == 1))
                    kvdst = AP(dest[:].tensor, dest[:].offset + o * KW,
                               [[2 * KFREE, 128], [2 * KW, KROWS], [1, KW]])
                    nc.scalar.activation(out=kvdst,
                                         in_=ps[:, 0:KFREE], func=AF.Identity,
                                         bias=bias[bn][:, o:o + 1])
                # +1-shifted fp16 copy so odd window offsets stay 4B-aligned
                nc.vector.tensor_copy(dest1[:, 0:2 * KFREE - 1], dest[:, 1:2 * KFREE])

            conv_kv("wk", "keypadpe", "bk", k_b, k_b1)

            # ---- attention j-loop, grouped by window row di ----
            num_ps = [pa.tile([128, NPOS], f32, tag=f"num{a}", name=f"num{a}")
                      for a in range(2)]
            den_ps = [pa.tile([128, NPOS], f32, tag=f"den{a}", name=f"den{a}")
                      for a in range(2)]

            RA = 2 * RQ   # merged (row, slot) dim: 20 metarows of 40

            def jmul(out_t, out_off, njs, kv_t, kv_off, in0=None):
                """out[j, ra, c] = in0 * kv[j-strided overlapping view];
                in0 defaults to q_b broadcast over j (stride 0)."""
                ov = AP(out_t[:].tensor, out_t[:].offset + out_off,
                        [[SBLK, 128], [NF, njs], [W, RA], [1, W]])
                kv = AP(kv_t[:].tensor, kv_t[:].offset + kv_off,
                        [[2 * KFREE, 128], [2, njs], [KW, RA], [1, W]])
                if in0 is None:
                    in0 = AP(q_b[:].tensor, q_b[:].offset,
                             [[NF, 128], [0, njs], [W, RA], [1, W]])
                nc.vector.tensor_mul(ov, in0, kv)

            def mm_rhs(src_t, a, jj):
                # e/p layout [j(7), ra(20), c(40)]: slice slot a of offset jj
                return AP(src_t[:].tensor, src_t[:].offset + jj * NF + a * W,
                          [[SBLK, 128], [2 * W, RQ], [1, W]])

            def acc_mm(ps_tile, src_t, a, di, last):
                """accumulate the 7 offsets of a row-block into one PSUM tile
                (per-j 128x400 fp16 matmuls; dense queue pipelines LDW+MM at
                ~169ns each when HAM-warm)."""
                for jj in range(KS):
                    nc.tensor.matmul(ps_tile[:], sb["ident"][:],
                                     mm_rhs(src_t, a, jj),
                                     start=(di == 0 and jj == 0),
                                     stop=(last and jj == KS - 1))

            # software-pipelined emission: the DVE/PE queues execute in order,
            # so s(di+1) is emitted BEFORE p(di) (which waits on exp(di)) to
            # keep DVE busy, and num(di-1) lands before den(di) on PE.
            s_ts, e_ts, p_ts = [], [], []

            def emit_s(di):
                s_t = sp.tile([128, SBLK], f16, tag="s", name=f"s{di}")
                e_t = sp.tile([128, SBLK], f16, tag="e", name=f"e{di}")
                s_ts.append(s_t)
                e_ts.append(e_t)
                jmul(s_t, 0, NJE, k_b, di * 2 * KW)
                jmul(s_t, NJE * NF, NJO, k_b1, di * 2 * KW)
                nc.scalar.activation(out=e_t[:], in_=s_t[:], func=AF.Exp)

            def emit_p(di):
                e_t = e_ts[di]
                p_t = sp.tile([128, SBLK], f16, tag="p", name=f"p{di}")
                p_ts.append(p_t)
                ev = AP(e_t[:].tensor, e_t[:].offset,
                        [[SBLK, 128], [NF, NJE], [W, RA], [1, W]])
                jmul(p_t, 0, NJE, v_b, di * 2 * KW, in0=ev)
                evo = AP(e_t[:].tensor, e_t[:].offset + NJE * NF,
                         [[SBLK, 128], [NF, NJO], [W, RA], [1, W]])
                jmul(p_t, NJE * NF, NJO, v_b1, di * 2 * KW, in0=evo)

            emit_s(0)
            # v conv emitted AFTER exp(0): the in-order ACT queue would
            # otherwise block exp(0) behind v evictions that wait on the
            # late keypad DMA.
            conv_kv("wv", "keypad", "bv", v_b, v_b1)
            # fuse conv, querype half: early PE work
            # layout [128, 1024]: o=0 at cols 0:400 (bank0), o=1 at 512:912
            fuse_ps = pk.tile([128, 1024], f32, tag="psbig", name="fuse_ps")
            for o in range(2):
                for k in range(2):
                    nc.tensor.matmul(fuse_ps[:, o * 512:o * 512 + NPOS],
                                     sb["wf"][:, k, o * 128:(o + 1) * 128],
                                     sb["querype"][:, k, :],
                                     start=(k == 0), stop=False)
            # PE warm-keeper: dummy matmuls on s(0) bridge the exp(0) gap so
            # HAM doesn't re-throttle before the first den matmuls.
            warm_ps = pq.tile([128, NPOS], f32, tag="psq", name="warmps")
            for jj in range(KS):
                nc.tensor.matmul(warm_ps[:], sb["ident"][:],
                                 mm_rhs(s_ts[0], 0, jj),
                                 start=True, stop=True)
            for di in range(1, KS):
                emit_s(di)
                for a in range(2):
                    acc_mm(den_ps[a], e_ts[di - 1], a, di - 1, False)
                emit_p(di - 1)
                for a in range(2):
                    acc_mm(num_ps[a], p_ts[di - 1], a, di - 1, False)
            for a in range(2):
                acc_mm(den_ps[a], e_ts[KS - 1], a, KS - 1, True)
            emit_p(KS - 1)
            for a in range(2):
                acc_mm(num_ps[a], p_ts[KS - 1], a, KS - 1, True)

            # ---- tail: normalize + vo conv + fuse conv, chunked for overlap ----
            NCH = 2
            NCHF = 4
            HCF = NPOS // NCHF
            HC = NPOS // NCH
            r_t = wp.tile([128, 2, NPOS], f32, tag="r_t")
            att = wp.tile([128, 2, NPOS], f16, tag="att")
            vo16 = wp.tile([128, 2, NPOS], f16, tag="vo16")
            out16 = wp.tile([128, 2, NPOS], f16, tag="out16")
            for ch in range(NCH):
                cs = slice(ch * HC, (ch + 1) * HC)
                for a in range(2):
                    nc.vector.reciprocal_approx_fast(r_t[:, a, cs], den_ps[a][:, cs])
            for ch in range(NCH):
                cs = slice(ch * HC, (ch + 1) * HC)
                for a in range(2):
                    nc.vector.tensor_mul(att[:, a, cs], num_ps[a][:, cs], r_t[:, a, cs])
            vops = [pa.tile([128, NPOS], f32, tag=f"num{o}", name="vops")
                    for o in range(2)]
            for ch in range(NCHF):
                cs = slice(ch * HCF, (ch + 1) * HCF)
                for o in range(2):
                    for a in range(2):
                        nc.tensor.matmul(vops[o][:, cs], sb["wo"][:, a, o * 128:(o + 1) * 128],
                                         att[:, a, cs], start=(a == 0), stop=(a == 1))
                    nc.scalar.activation(out=vo16[:, o, cs], in_=vops[o][:, cs],
                                         func=AF.Identity, bias=bias["bo"][:, o:o + 1])
                for o in range(2):
                    for k in range(2):
                        nc.tensor.matmul(
                            fuse_ps[:, o * 512 + ch * HCF:o * 512 + (ch + 1) * HCF],
                            sb["wf"][:, 2 + k, o * 128:(o + 1) * 128],
                            vo16[:, k, cs], start=False, stop=(k == 1))
                    fslice = fuse_ps[:, o * 512 + ch * HCF:o * 512 + (ch + 1) * HCF]
                    if o == 0:
                        nc.scalar.copy(out16[:, o, cs], fslice)
                    else:
                        nc.vector.tensor_copy(out16[:, o, cs], fslice)
                nc.sync.dma_start(out=d_vo[:, :, cs], in_=vo16[:, :, cs])
                nc.gpsimd.dma_start(out=d_out[:, :, cs], in_=out16[:, :, cs])

    nc.compile()
    _CACHE["nc"] = nc
    return nc


def _in_maps(key, query, Wq, bq, Wk, bk, Wv, bv, Wo, bo, Wf):
    pe_q, pe_k = _pe_constants()
    keypad_full = np.pad(key, ((0, 0), (0, 0), (PAD, PAD), (PAD, PAD)))
    querype_full = (query + pe_q[None]).astype(np.float16)
    keypadpe_full = (keypad_full + pe_k[None]).astype(np.float16)
    keypad_full = keypad_full.astype(np.float16)
    def wdev(w, nk, scale=1.0):  # (out, in) -> [128, nk, 256] fp16
        return np.ascontiguousarray(
            (w.T * scale).reshape(nk, 128, 256).transpose(1, 0, 2)).astype(np.float16)

    wqT = wdev(Wq, 2, SCALING)
    wkT = wdev(Wk, 2)
    wvT = wdev(Wv, 2)
    woT = wdev(Wo, 2)
    wfT = wdev(Wf, 4)
    biases = np.stack([(bq * SCALING), bk, bv, bo], 0).reshape(4, 2, 128)
    biases = np.ascontiguousarray(biases.reshape(8, 128).T).astype(np.float32)
    ident = np.eye(128, dtype=np.float16)

    def part16(arr, npos):  # (C, rows*cols) -> (128, 2, npos) fp16
        return np.ascontiguousarray(
            arr.reshape(2, 128, npos).transpose(1, 0, 2)).astype(np.float16)

    maps = []
    for b in range(B):
        for q in range(NQ):
            r0 = RQ * q
            m = {
                "querype": part16(querype_full[b, :, r0:r0 + RQ, :].reshape(C, NPOS), NPOS),
                "keypad": part16(keypad_full[b, :, r0:r0 + KROWS, :].reshape(C, KFREE), KFREE),
                "keypadpe": part16(keypadpe_full[b, :, r0:r0 + KROWS, :].reshape(C, KFREE), KFREE),
                "wq": wqT, "wk": wkT, "wv": wvT, "wo": woT, "wf": wfT,
                "biases": biases, "ident": ident,
            }
            maps.append(m)
    return maps


def kernel(key, query, Wq, bq, Wk, bk, Wv, bv, Wo, bo, Wf, _trace=False):
    from concourse.bass_utils import run_bass_kernel_spmd

    args = [np.asarray(a, dtype=np.float32) for a in
            (key, query, Wq, bq, Wk, bk, Wv, bv, Wo, bo, Wf)]
    nc = _build_module()
    maps = _in_maps(*args)
    res = run_bass_kernel_spmd(nc, maps, list(range(8)), trace=_trace)
    _CACHE["last_res"] = res

    out = np.zeros((B, C, H, W), dtype=np.float32)
    vo = np.zeros((B, C, H, W), dtype=np.float32)
    for b in range(B):
        for q in range(NQ):
            r = res.results[b * NQ + q]
            r0 = RQ * q
            out[b, :, r0:r0 + RQ, :] = (
                r["out_part"].transpose(1, 0, 2).reshape(C, RQ, W).astype(np.float32))
            vo[b, :, r0:r0 + RQ, :] = (
                r["vo_part"].transpose(1, 0, 2).reshape(C, RQ, W).astype(np.float32))
    return out, vo
